# revision 1
# baseline (speedup 1.0000x reference)
"""CoAtNet transformer block kernel for Trainium2 (8 NeuronCores).

Strategy:
  - Data-parallel over batch: 64 images -> 8 per core, no collectives.
  - Channel-major activation layout [C, N] on chip (x arrives as (C, H*W)).
  - All matmuls in bf16 (fp32 PSUM accumulation); LN stats / residuals fp32.
  - LayerNorm gamma/beta folded into QKV weights host-side. Q/K projections
    run on raw (un-normalized) bf16 x so they never wait on the LN stats
    chain; the per-token (mean, rstd) correction is applied to the PSUM
    result as rstd*psum + mr*colsum(w) (+bias) on the vector/scalar engines.
  - Attention computed transposed (simT[m, n] = k@q.T + biasT) so softmax
    normalization is a column sum obtained for free from a ones-column in
    the V matmul; bias applied as precomputed exp(biasT) multiplier.
  - Batches processed in pairs so the moving free dim is 392 (hides
    LDWEIGHTS under the matmul stream); per-head sim matmuls packed two
    heads at a time into disjoint PE row groups.
  - FFN: per 128-wide h1 chunk, gelu then immediately accumulate into six
    persistent output PSUM banks (no full h1 materialization).
"""

import numpy as np
import ml_dtypes

H = 14
W = 14
C = 768
HEADS = 12
EXPAND = 4
N = H * W  # 196
B = 64
NCORES = 8
BPC = B // NCORES  # 8 batches per core
DH = C // HEADS  # 64
KC = C // 128  # 6 chunks of 128 channels
F = C * EXPAND  # 3072
KF = F // 128  # 24
NPAIR = 2 * N  # 392
M0, M1 = 128, N - 128  # token chunks 128 + 68
MCHUNKS = ((0, M0), (M0, M1))


def _relative_indices():
    gy, gx = np.meshgrid(np.arange(H), np.arange(W), indexing="ij")
    py, px = gy.reshape(-1), gx.reshape(-1)
    rel_y = py[None, :] - py[:, None] + H
    rel_x = px[None, :] - px[:, None] + W
    return rel_y * W + rel_x  # (N, N) int


_SIM_NO_GELU = False  # CoreSim lacks Gelu; debug harness flips this


def _build_bass():
    import concourse.bacc as bacc
    import concourse.mybir as mybir
    import concourse.tile as tile

    f32 = mybir.dt.float32
    bf16 = mybir.dt.bfloat16
    AF = mybir.ActivationFunctionType
    OP = mybir.AluOpType

    nc = bacc.Bacc("TRN2")

    # ---- DRAM parameters (per core) ----
    x_in = nc.declare_dram_parameter("x", [BPC, C, N], bf16, isOutput=False)
    wq_d = nc.declare_dram_parameter("wq", [C, C], bf16, isOutput=False)
    wk_d = nc.declare_dram_parameter("wk", [C, C], bf16, isOutput=False)
    wv_d = nc.declare_dram_parameter("wv", [C, C], bf16, isOutput=False)
    wo_d = nc.declare_dram_parameter("wo", [C, C], bf16, isOutput=False)
    w1_d = nc.declare_dram_parameter("w1", [C, F], bf16, isOutput=False)
    w2_d = nc.declare_dram_parameter("w2", [F, C], bf16, isOutput=False)
    bq_d = nc.declare_dram_parameter("bq", [C], f32, isOutput=False)
    bk_d = nc.declare_dram_parameter("bk", [C], f32, isOutput=False)
    bo_d = nc.declare_dram_parameter("bo", [C], f32, isOutput=False)
    b1_d = nc.declare_dram_parameter("b1", [F], f32, isOutput=False)
    b2_d = nc.declare_dram_parameter("b2", [C], f32, isOutput=False)
    wsq_d = nc.declare_dram_parameter("wsq", [C], f32, isOutput=False)
    wsk_d = nc.declare_dram_parameter("wsk", [C], f32, isOutput=False)
    # exp(biasT) per head, token-chunked: [128, HEADS, N] and [68, HEADS, N]
    eb0_d = nc.declare_dram_parameter("eb0", [M0, HEADS, N], bf16, isOutput=False)
    eb1_d = nc.declare_dram_parameter("eb1", [M1, HEADS, N], bf16, isOutput=False)
    # selector matrices broadcasting softmax-reciprocal rows (0,32)/(64,96)
    # of rec4 to row halves 0-63 / 64-127 via the PE
    sel_d = nc.declare_dram_parameter("sel", [2, 128, 128], bf16, isOutput=False)
    out_d = nc.declare_dram_parameter("out", [BPC, C, N], f32, isOutput=True)

    def ld(pool, name, dram, shape, pat):
        t = pool.tile(shape, dram.dtype, name=name)
        nc.sync.dma_start(t[:], dram.ap().rearrange(pat, p=128) if pat else dram.ap())
        return t

    with tile.TileContext(nc) as tc:
        with (
            tc.tile_pool(name="wpool", bufs=1) as wpool,
            tc.tile_pool(name="acts", bufs=1) as acts,
            tc.tile_pool(name="xio", bufs=2) as xio,
            tc.tile_pool(name="small", bufs=2) as small,
            tc.tile_pool(name="psum", bufs=1, space="PSUM") as pp,
        ):
            def load_x(pair):
                b0 = 2 * pair
                t = xio.tile([128, 2 * KC, N], bf16, name="xf", tag="xf", bufs=3)
                nc.sync.dma_start(
                    t[:],
                    x_in.ap()[b0 : b0 + 2].rearrange(
                        "b (ko p) n -> p (b ko) n", p=128
                    ),
                )
                return t

            # ---- DMA issue order = arrival order: x for the first pairs
            # first, then weights in order of first use.
            xfs = {0: load_x(0)}
            wq_sb = ld(wpool, "wq_sb", wq_d, [128, KC, C], "(ko p) m -> p ko m")
            wk_sb = ld(wpool, "wk_sb", wk_d, [128, KC, C], "(ko p) m -> p ko m")
            bq_sb = ld(wpool, "bq_sb", bq_d, [128, KC], "(mo p) -> p mo")
            bk_sb = ld(wpool, "bk_sb", bk_d, [128, KC], "(mo p) -> p mo")
            wsq_sb = ld(wpool, "wsq_sb", wsq_d, [128, KC], "(mo p) -> p mo")
            wsk_sb = ld(wpool, "wsk_sb", wsk_d, [128, KC], "(mo p) -> p mo")
            wv_sb = ld(wpool, "wv_sb", wv_d, [128, KC, C], "(ko p) m -> p ko m")
            xfs[1] = load_x(1)
            eb0_sb = ld(wpool, "eb0_sb", eb0_d, [M0, HEADS, N], None)
            eb1_sb = ld(wpool, "eb1_sb", eb1_d, [M1, HEADS, N], None)
            bo_sb = ld(wpool, "bo_sb", bo_d, [128, KC], "(mo p) -> p mo")
            sel_sb = ld(wpool, "sel_sb", sel_d, [128, 2, 128], "t p m -> p t m")
            wo_sb = ld(wpool, "wo_sb", wo_d, [128, KC, C], "(ko p) m -> p ko m")
            b1_sb = ld(wpool, "b1_sb", b1_d, [128, KF], "(mo p) -> p mo")
            w1_sb = ld(wpool, "w1_sb", w1_d, [128, KC, F], "(ko p) m -> p ko m")
            b2_sb = ld(wpool, "b2_sb", b2_d, [128, KC], "(mo p) -> p mo")
            w2_sb = ld(wpool, "w2_sb", w2_d, [128, KF, C], "(ko p) m -> p ko m")
            ones_sb = wpool.tile([128, 1], bf16, name="ones_sb")
            nc.vector.memset(ones_sb[:], 1.0)
            eps_sb = wpool.tile([1, 1], f32, name="eps_sb")
            nc.vector.memset(eps_sb[:], 1e-5)
            zero_sb = wpool.tile([128, 1], f32, name="zero_sb")
            nc.vector.memset(zero_sb[:], 0.0)

            eb_sb = (eb0_sb, eb1_sb)
            NP2 = BPC // 2

            def stage_a1(pair):
                xf = xfs.pop(pair)
                # pair view: [128, KC, 2, N] (chunk-major, batch inner)
                xfv = xf.rearrange("p (b k) n -> p k b n", b=2)
                # ---- LN stats matmuls (x already bf16) ----
                s_ps = pp.tile([1, NPAIR], f32, name="s_ps", tag="st0")
                q_ps = pp.tile([1, NPAIR], f32, name="q_ps", tag="st1")
                for k in range(KC):
                    xsq = small.tile([128, 2, N], bf16, name="xsq", tag="xsq")
                    nc.gpsimd.tensor_mul(xsq[:], xfv[:, k], xfv[:, k])
                    nc.tensor.matmul(
                        s_ps[:], ones_sb[:], xfv[:, k],
                        start=(k == 0), stop=(k == KC - 1),
                    )
                    nc.tensor.matmul(
                        q_ps[:], ones_sb[:], xsq[:],
                        start=(k == 0), stop=(k == KC - 1),
                    )

                # ---- LN stats chain (off PE critical path) ----
                mu = small.tile([1, NPAIR], f32, name="mu", tag="mu", bufs=1)
                nc.vector.tensor_scalar_mul(mu[:], s_ps[:], 1.0 / C)
                var = small.tile([1, NPAIR], f32, name="var", tag="var", bufs=1)
                # var = (mu * -mu) + sq/C
                nc.vector.scalar_tensor_tensor(
                    var[:], mu[:], -1.0, mu[:], OP.mult, OP.mult
                )
                nc.vector.scalar_tensor_tensor(
                    var[:], q_ps[:], 1.0 / C, var[:], OP.mult, OP.add
                )
                std = small.tile([1, NPAIR], f32, name="std", tag="std", bufs=1)
                nc.scalar.activation(
                    std[:], var[:], AF.Sqrt, bias=eps_sb[:], scale=1.0
                )
                rstd = small.tile([1, NPAIR], f32, name="rstd", tag="rstd", bufs=1)
                nc.vector.reciprocal(rstd[:], std[:])
                mr = small.tile([1, NPAIR], f32, name="mr", tag="mr", bufs=1)
                nc.vector.scalar_tensor_tensor(
                    mr[:], mu[:], -1.0, rstd[:], OP.mult, OP.mult
                )
                rstd_b = small.tile(
                    [128, NPAIR], f32, name="rstd_b", tag="rstd_b", bufs=1
                )
                nc.gpsimd.partition_broadcast(rstd_b[:], rstd[:])
                mr_b = small.tile([128, NPAIR], f32, name="mr_b", tag="mr_b", bufs=1)
                nc.gpsimd.partition_broadcast(mr_b[:], mr[:])
                rstd_bv = rstd_b.rearrange("p (b n) -> p b n", b=2)
                mr_bv = mr_b.rearrange("p (b n) -> p b n", b=2)
                return dict(
                    xfv=xfv, rstd_b=rstd_b, mr_b=mr_b,
                    rstd_bv=rstd_bv, mr_bv=mr_bv,
                )

            def stage_a2(ctx):
                xfv = ctx["xfv"]
                rstd_b, mr_b = ctx["rstd_b"], ctx["mr_b"]
                rstd_bv, mr_bv = ctx["rstd_bv"], ctx["mr_bv"]

                # ---- Q/K projections from RAW x_bf; LN applied post-hoc:
                #      qT = rstd*(w'.T@x) + mr*colsum(w') + b' ----
                qT = acts.tile([128, KC, NPAIR], bf16, name="qT", tag="qT")
                kT = acts.tile([128, KC, NPAIR], bf16, name="kT", tag="kT")
                for dst, w_sb, ws_sb, bias_sb in (
                    (qT, wq_sb, wsq_sb, bq_sb),
                    (kT, wk_sb, wsk_sb, bk_sb),
                ):
                    for m in range(KC):
                        ps = pp.tile(
                            [128, NPAIR], f32, name="ps_qk", tag="mm", bufs=3
                        )
                        for k in range(KC):
                            nc.tensor.matmul(
                                ps[:],
                                w_sb[:, k, 128 * m : 128 * (m + 1)],
                                xfv[:, k],
                                start=(k == 0),
                                stop=(k == KC - 1),
                            )
                        t1 = small.tile([128, NPAIR], f32, name="t1", tag="t1")
                        nc.vector.tensor_mul(t1[:], ps[:], rstd_b[:])
                        t2 = small.tile([128, NPAIR], bf16, name="t2", tag="t2")
                        nc.vector.scalar_tensor_tensor(
                            t2[:], mr_b[:], ws_sb[:, m : m + 1], t1[:],
                            OP.mult, OP.add,
                        )
                        nc.scalar.activation(
                            dst[:, m, :], t2[:], AF.Identity,
                            bias=bias_sb[:, m : m + 1], scale=1.0,
                        )

                # ---- xn = x*rstd + mr (bf16; only feeds the V projection) ----
                xn = acts.tile([128, KC, NPAIR], bf16, name="xn", tag="xn")
                xnv = xn.rearrange("p k (b n) -> p k b n", b=2)
                for k in range(KC):
                    t = small.tile([128, NPAIR], f32, name="t_ln", tag="t1")
                    nc.vector.tensor_mul(
                        t.rearrange("p (b n) -> p b n", b=2), xfv[:, k], rstd_bv
                    )
                    nc.vector.tensor_add(
                        xnv[:, k], t.rearrange("p (b n) -> p b n", b=2), mr_bv
                    )
                ctx.update(qT=qT, kT=kT, xn=xn)

            def stage_b(pair, ctx):
                b0 = 2 * pair
                xfv, qT, kT, xn = ctx["xfv"], ctx["qT"], ctx["kT"], ctx["xn"]
                if pair + 2 < NP2:
                    xfs[pair + 2] = load_x(pair + 2)

                # ---- V projection (token-major, per batch, 65-strided heads);
                #      emitted per (j, ci, s) so j=1 chunks can interleave
                #      into j=0's attention stream as PE fillers ----
                v_sb = [[None, None], [None, None]]

                def emit_vproj(j, ci, s):
                    mstart, mlen = MCHUNKS[ci]
                    vt = v_sb[j][ci]
                    if vt is None:
                        vt = acts.tile(
                            [128, HEADS, DH + 1], bf16, name=f"v_{j}_{ci}",
                            tag=f"v_{j}_{ci}",
                        )
                        v_sb[j][ci] = vt
                        nc.vector.memset(vt[:mlen, :, DH : DH + 1], 1.0)
                    pv = pp.tile([128, 384], f32, name="pv", tag="mm", bufs=3)
                    for k in range(KC):
                        nc.tensor.matmul(
                            pv[:mlen],
                            xn[:, k, j * N + mstart : j * N + mstart + mlen],
                            wv_sb[:, k, 384 * s : 384 * (s + 1)],
                            start=(k == 0),
                            stop=(k == KC - 1),
                        )
                    nc.scalar.activation(
                        vt[:mlen, 6 * s : 6 * (s + 1), 0:DH],
                        pv[:mlen].rearrange("p (h d) -> p h d", h=6),
                        AF.Identity, bias=zero_sb[:mlen], scale=1.0,
                    )

                # ---- attention: head-pairs packed into PE row groups;
                #      emission software-pipelined one pair ahead; softmax
                #      reciprocals batched 4 heads at a time ----
                OT = acts.tile([128, KC, NPAIR], bf16, name="OT", tag="OT")
                ybf = acts.tile([128, KC, NPAIR], bf16, name="ybf", tag="ybf")

                def emit_sims(j, hp):
                    cb = j * N
                    sims, ets = [], []
                    for hh in range(2):  # heads 2*hp, 2*hp+1
                        prow = 64 * hh
                        sim = pp.tile(
                            [128, 2, N], f32, name="sim", tag=f"st{hh}"
                        )
                        for ci, (mstart, mlen) in enumerate(MCHUNKS):
                            nc.tensor.matmul(
                                sim[:mlen, ci],
                                kT[prow : prow + 64, hp,
                                   cb + mstart : cb + mstart + mlen],
                                qT[prow : prow + 64, hp, cb : cb + N],
                                start=True, stop=True,
                            )
                        et = small.tile(
                            [128, 2, N], bf16, name="et", tag=f"et{hh}"
                        )
                        nc.scalar.activation(
                            et[:, 0], sim[:, 0], AF.Exp, bias=0.0, scale=1.0
                        )
                        nc.scalar.activation(
                            et[:M1, 1], sim[:M1, 1], AF.Exp, bias=0.0, scale=1.0
                        )
                        for ci, (mstart, mlen) in enumerate(MCHUNKS):
                            nc.vector.tensor_mul(
                                et[:mlen, ci], et[:mlen, ci],
                                eb_sb[ci][:mlen, 2 * hp + hh, :],
                            )
                        sims.append(sim)
                        ets.append(et)
                    return ets

                def emit_omm_mms(j, hp, ets, dsum):
                    # two heads packed into one PSUM bank: [128, 2, N]
                    opp = pp.tile(
                        [128, 2, N], f32, name="opp",
                        tag="po01" if hp % 2 == 0 else "po23",
                    )
                    out = []
                    for hh in range(2):
                        h = 2 * hp + hh
                        g = h % 4
                        for ci, (mstart, mlen) in enumerate(MCHUNKS):
                            nc.tensor.matmul(
                                opp[: DH + 1, hh],
                                v_sb[j][ci][:mlen, h, :],
                                ets[hh][:mlen, ci],
                                start=(ci == 0), stop=(ci == 1),
                            )
                        nc.scalar.activation(
                            dsum[32 * g : 32 * g + 1, :], opp[DH : DH + 1, hh],
                            AF.Identity, bias=zero_sb[:1], scale=1.0,
                        )
                        out.append((opp, hh))
                    return out

                def emit_epi(j, hp, ops2, dsum, rec4):
                    cb = j * N
                    t = hp % 2
                    # denominators sit on partitions 0/32/64/96 (32-aligned
                    # base-partition requirement); one reciprocal per 64-row
                    # half covers this hp's two heads (dead rows memset to
                    # 1.0). A selector matmul broadcasts rows (0,32) or
                    # (64,96) to row halves — no gpsimd partition tricks.
                    with nc.allow_low_precision(reason="bf16 softmax"):
                        nc.vector.reciprocal(
                            rec4[64 * t : 64 * t + 64, :],
                            dsum[64 * t : 64 * t + 64, :],
                        )
                    rps = pp.tile([128, N], f32, name="rps", tag="mm",
                                  bufs=3)
                    nc.tensor.matmul(
                        rps[:], sel_sb[64 * t : 64 * t + 64, t, :],
                        rec4[64 * t : 64 * t + 64, :],
                        start=True, stop=True,
                    )
                    rsb = small.tile([128, N], bf16, name="rsb", tag="rsb")
                    nc.scalar.activation(
                        rsb[:], rps[:], AF.Identity, bias=zero_sb[:],
                        scale=1.0,
                    )
                    for hh in range(2):
                        opp, phh = ops2[hh]
                        nc.vector.tensor_mul(
                            OT[64 * hh : 64 * hh + 64, hp, cb : cb + N],
                            opp[0:DH, phh],
                            rsb[64 * hh : 64 * hh + 64, :],
                        )

                def attention_j(j, fillers):
                    fq = list(fillers)

                    def fill():
                        if fq:
                            fq.pop(0)()

                    prev = None
                    dsum = None
                    rec4 = None
                    for hp in range(HEADS // 2):
                        ets = emit_sims(j, hp)
                        fill()
                        if prev is not None:
                            php, pets = prev
                            if php % 2 == 0:
                                dsum = small.tile(
                                    [128, N], f32, name="dsum", tag="dsum"
                                )
                                nc.vector.memset(dsum[:], 1.0)
                                rec4 = small.tile(
                                    [128, N], bf16, name="rec4", tag="rec4"
                                )
                            ops2 = emit_omm_mms(j, php, pets, dsum)
                            emit_epi(j, php, ops2, dsum, rec4)
                        prev = (hp, ets)
                    php, pets = prev
                    ops2 = emit_omm_mms(j, php, pets, dsum)
                    emit_epi(j, php, ops2, dsum, rec4)
                    while fq:
                        fq.pop(0)()

                def emit_outproj(j, m):
                    cb = j * N
                    po = pp.tile([128, N], f32, name="po", tag="mm", bufs=3)
                    for k in range(KC):
                        nc.tensor.matmul(
                            po[:],
                            wo_sb[:, k, 128 * m : 128 * (m + 1)],
                            OT[:, k, cb : cb + N],
                            start=(k == 0),
                            stop=(k == KC - 1),
                        )
                    with nc.allow_low_precision(reason="bf16 residual-1"):
                        nc.vector.scalar_tensor_tensor(
                            ybf[:, m, cb : cb + N],
                            po[:],
                            bo_sb[:, m : m + 1],
                            xfv[:, m, j],
                            OP.add, OP.add,
                        )
                # V proj j=0 up front; j=1 chunks + first sims feed the gaps
                for ci in range(2):
                    for s in range(2):
                        emit_vproj(0, ci, s)
                attention_j(
                    0,
                    [lambda ci=ci, s=s: emit_vproj(1, ci, s)
                     for ci in range(2) for s in range(2)],
                )
                attention_j(
                    1,
                    [lambda m=m: emit_outproj(0, m) for m in range(KC)],
                )

                o32 = xio.tile([128, 2 * KC, N], f32, name="o32", tag="o32", bufs=1)
                o32v = o32.rearrange("p (b k) n -> p k b n", b=2)
                def ffn_core_j(j):
                    # FFN1 materializes gelu(h1) in SBUF; FFN2 runs as two
                    # 3-chunk passes so each PSUM bank hosts exactly one
                    # accumulation group at a time. Pass-1 banks (st0/st1/
                    # po45) are free during the attention epilogue, so the
                    # FFN starts with zero WAR stall; by pass 2 the
                    # epilogue has drained po01/po23.
                    cb = j * N
                    h1s = acts.tile([128, KF, N], bf16, name="h1s", tag="h1s")
                    tags1 = ("st0", "st1", "po45")
                    tags2 = ("po01", "po23", "po45")
                    p_a = [
                        pp.tile([128, N], f32, name=f"pfa{o}", tag=tags1[o])
                        for o in range(3)
                    ]
                    for mf in range(KF):
                        p1 = pp.tile([128, N], f32, name="p1", tag="mm", bufs=3)
                        for k in range(KC):
                            nc.tensor.matmul(
                                p1[:],
                                w1_sb[:, k, 128 * mf : 128 * (mf + 1)],
                                ybf[:, k, cb : cb + N],
                                start=(k == 0),
                                stop=(k == KC - 1),
                            )
                        nc.scalar.activation(
                            h1s[:, mf, :], p1[:],
                            AF.Identity if _SIM_NO_GELU else AF.Gelu,
                            bias=b1_sb[:, mf : mf + 1],
                            scale=1.0,
                        )
                        for o in range(3):
                            nc.tensor.matmul(
                                p_a[o][:],
                                w2_sb[:, mf, 128 * o : 128 * (o + 1)],
                                h1s[:, mf, :],
                                start=(mf == 0),
                                stop=(mf == KF - 1),
                            )
                    p_b = [
                        pp.tile([128, N], f32, name=f"pfb{o}", tag=tags2[o])
                        for o in range(3)
                    ]
                    for o in range(3):
                        for mf in range(KF):
                            nc.tensor.matmul(
                                p_b[o][:],
                                w2_sb[:, mf, 128 * (o + 3) : 128 * (o + 4)],
                                h1s[:, mf, :],
                                start=(mf == 0),
                                stop=(mf == KF - 1),
                            )
                    return p_a + p_b

                def ffn_store_j(j, pps):
                    cb = j * N
                    # residual 2 + per-chunk store
                    for o in range(KC):
                        nc.vector.scalar_tensor_tensor(
                            o32v[:, o, j],
                            pps[o][:],
                            b2_sb[:, o : o + 1],
                            ybf[:, o, cb : cb + N],
                            OP.add, OP.add,
                        )
                        nc.sync.dma_start(
                            out_d.ap()[
                                b0 + j : b0 + j + 1, 128 * o : 128 * (o + 1)
                            ].rearrange("b (ko p) n -> p (b ko n)", p=128),
                            o32v[:, o, j],
                        )

                # FFN j=0 PE work hides attention j=1's softmax epilogue;
                # next pair's stats/LN chain and Q/K projections are spliced
                # into the FFN/out-proj stream so the pair boundary has no
                # engine flush; out-proj j=1 follows once OT j=1 is done.
                pps0 = ffn_core_j(0)
                ffn_store_j(0, pps0)
                nctx = stage_a1(pair + 1) if pair + 1 < NP2 else None
                for m in range(KC):
                    emit_outproj(1, m)
                if nctx is not None:
                    stage_a2(nctx)
                pps1 = ffn_core_j(1)
                ffn_store_j(1, pps1)
                return nctx

            ctx = stage_a1(0)
            stage_a2(ctx)
            for pair in range(NP2):
                ctx = stage_b(pair, ctx)
    nc.finalize()
    return nc


_CACHE = {}


def prepare_in_maps(inputs):
    x = np.asarray(inputs["x"], dtype=np.float32)  # (64, 768, 14, 14)
    ln_g = np.asarray(inputs["ln_g"], dtype=np.float32)
    ln_b = np.asarray(inputs["ln_b"], dtype=np.float32)
    wq = np.asarray(inputs["wq"], dtype=np.float32)
    bq = np.asarray(inputs["bq"], dtype=np.float32)
    wk = np.asarray(inputs["wk"], dtype=np.float32)
    bk = np.asarray(inputs["bk"], dtype=np.float32)
    wv = np.asarray(inputs["wv"], dtype=np.float32)
    bv = np.asarray(inputs["bv"], dtype=np.float32)
    wo = np.asarray(inputs["wo"], dtype=np.float32)
    bo = np.asarray(inputs["bo"], dtype=np.float32)
    w1 = np.asarray(inputs["w1"], dtype=np.float32)
    b1 = np.asarray(inputs["b1"], dtype=np.float32)
    w2 = np.asarray(inputs["w2"], dtype=np.float32)
    b2 = np.asarray(inputs["b2"], dtype=np.float32)
    rel_bias = np.asarray(inputs["rel_bias"], dtype=np.float32)

    bf = ml_dtypes.bfloat16

    # Fold LayerNorm gamma into QKV weights, beta into their biases.
    wqp_f = ln_g[:, None] * wq
    wkp_f = ln_g[:, None] * wk
    wvp_f = ln_g[:, None] * wv
    bqp = (ln_b @ wq + bq).astype(np.float32)
    bkp = (ln_b @ wk + bk).astype(np.float32)
    bvp = (ln_b @ wv + bv).astype(np.float32)
    # V bias commutes through softmax (rows sum to 1): fold into out-proj bias.
    bop = (bo + bvp @ wo).astype(np.float32)
    # Column sums of the folded Q/K weights for the post-hoc mean correction.
    # Use the bf16-rounded weights so the correction matches the matmul.
    wqp = wqp_f.astype(bf)
    wkp = wkp_f.astype(bf)
    wsq = wqp.astype(np.float32).sum(axis=0).astype(np.float32)
    wsk = wkp.astype(np.float32).sum(axis=0).astype(np.float32)

    # Relative position bias, transposed per head, exponentiated.
    rel_idx = _relative_indices()
    bias = rel_bias[:, rel_idx]  # (HEADS, N, N) : bias[h, n, m]
    ebT = np.exp(bias.transpose(0, 2, 1))  # (HEADS, m, n)
    eb_m = ebT.transpose(1, 0, 2)  # (m, HEADS, n)
    eb0 = np.ascontiguousarray(eb_m[:M0]).astype(bf)
    eb1 = np.ascontiguousarray(eb_m[M0:]).astype(bf)

    # PE-broadcast selectors: out = sel[t].T @ rec4 replicates rec4 row
    # (0, 32) -> output halves for t=0, rows (64, 96) for t=1.
    sel = np.zeros((2, 128, 128), dtype=np.float32)
    sel[0, 0, 0:64] = 1.0
    sel[0, 32, 64:128] = 1.0
    sel[1, 64, 0:64] = 1.0
    sel[1, 96, 64:128] = 1.0
    sel = sel.astype(bf)

    common = {
        "wq": wqp, "wk": wkp, "wv": wvp_f.astype(bf),
        "wo": wo.astype(bf), "w1": w1.astype(bf), "w2": w2.astype(bf),
        "bq": bqp, "bk": bkp, "bo": bop,
        "b1": b1.astype(np.float32), "b2": b2.astype(np.float32),
        "wsq": wsq, "wsk": wsk,
        "eb0": eb0, "eb1": eb1,
        "sel": sel,
    }

    x_flat = x.reshape(B, C, N)
    in_maps = []
    for c in range(NCORES):
        m = dict(common)
        m["x"] = np.ascontiguousarray(x_flat[c * BPC : (c + 1) * BPC]).astype(bf)
        in_maps.append(m)
    return in_maps


def kernel(**inputs):
    import sys

    if "/opt/trn_rl_repo" not in sys.path:
        sys.path.insert(0, "/opt/trn_rl_repo")
    from concourse.bass_utils import run_bass_kernel_spmd

    in_maps = prepare_in_maps(inputs)

    if "nc" not in _CACHE:
        _CACHE["nc"] = _build_bass()
    nc = _CACHE["nc"]

    res = run_bass_kernel_spmd(nc, in_maps, core_ids=list(range(NCORES)))
    _CACHE["last_res"] = res
    outs = [r["out"] for r in res.results]
    full = np.concatenate(outs, axis=0)  # (64, 768, 196)
    return full.reshape(B, C, H, W).astype(np.float32)



# revision 12
# speedup vs baseline: 1.1046x; 1.1046x over previous
"""CoAtNet transformer block kernel for Trainium2 (8 NeuronCores).

Strategy:
  - Data-parallel over batch: 64 images -> 8 per core, no collectives.
  - Channel-major activation layout [C, N] on chip (x arrives as (C, H*W)).
  - Matmuls bf16 except the FFN, which runs fp8e4 (TRN E4M3, max 240)
    with DoubleRow perf mode (2 contraction chunks per instruction, 2x
    PE throughput). Weights are pre-scaled x64 host-side so fp8 values
    sit in the normal range; the 1/64 unscale folds into the existing
    PSUM-read epilogues. QKV weights (bf16) carry the same x64 (exact
    in bf16) so one rstd/64 factor serves every LN epilogue.
  - LayerNorm gamma/beta folded into QKV weights host-side. Q/K/V all
    project from RAW bf16 x so no projection waits on the LN stats
    chain; corrections are applied post-hoc:
      q = rstd64*psum + mr64*colsum(w') + b  (channel-major, vector ops)
      v = rstd64*(psum + (-mu) (x) colsum(wv'))  (token-major: rank-1
          -mu*colsum accumulated into PSUM by a 1-row matmul, rstd64
          applied as a per-partition activation scale via a tiny PE
          transpose of the rstd row)
  - Attention computed transposed; the relative-position bias is
    accumulated into the sim PSUM by an identity-weight matmul (no
    vector multiply, exp reads PSUM directly). V carries 64 ones
    columns so attn@v lands numerator (rows 0:64) and denominator
    (rows 64:128) in one PSUM bank; softmax reciprocal runs
    reciprocal_approx_fast on the single denominator row, and a
    selector matmul broadcasts both heads' reciprocal rows for the
    final multiply.
  - FFN: 24 h1 chunks gelu'd straight into fp8; two 3-output-chunk
    passes of DoubleRow accumulation into persistent PSUM banks.
  - Pair pipeline: stats for pair p+1 (sum matmuls first, sumsq after
    gpsimd squares) and Q/K projections are spliced into pair p's
    FFN/out-proj stream.
"""

import numpy as np
import ml_dtypes

H = 14
W = 14
C = 768
HEADS = 12
EXPAND = 4
N = H * W  # 196
B = 64
NCORES = 8
BPC = B // NCORES  # 8 batches per core
DH = C // HEADS  # 64
KC = C // 128  # 6 chunks of 128 channels
F = C * EXPAND  # 3072
KF = F // 128  # 24
NPAIR = 2 * N  # 392
M0, M1 = 128, N - 128  # token chunks 128 + 68
MCHUNKS = ((0, M0), (M0, M1))
WS = 64.0  # weight pre-scale (exact in bf16, lifts fp8 out of denormals)

# fp8 scope flags (DoubleRow perf mode). Sim'd rel-err: both on 1.86e-2,
# ffn2 only 1.31e-2, bf16 2.8e-3 (gate 2e-2).
F8_FFN1 = True
F8_FFN2 = True
# bisect probes (module-level monkeypatchable)
# NOTE: _SPLICE=True faults HW (NRT_EXEC_UNIT_UNRECOVERABLE) — emitting
# next-pair stats/qk inside pair p's FFN stream triggers it; sequential
# emission works.
_SPLICE = False  # emit next-pair stats/qk inside pair p's FFN stream
_DO_FFN = True   # run the FFN (else copy ybf to out)
_DO_ATTN = True  # run attention internals (else memset OT)


def _relative_indices():
    gy, gx = np.meshgrid(np.arange(H), np.arange(W), indexing="ij")
    py, px = gy.reshape(-1), gx.reshape(-1)
    rel_y = py[None, :] - py[:, None] + H
    rel_x = px[None, :] - px[:, None] + W
    return rel_y * W + rel_x  # (N, N) int


_SIM_NO_GELU = False  # CoreSim lacks Gelu; debug harness flips this


def _build_bass():
    import concourse.bacc as bacc
    import concourse.mybir as mybir
    import concourse.tile as tile

    f32 = mybir.dt.float32
    bf16 = mybir.dt.bfloat16
    f8 = mybir.dt.float8e4
    AF = mybir.ActivationFunctionType
    OP = mybir.AluOpType
    DR = mybir.MatmulPerfMode.DoubleRow

    nc = bacc.Bacc("TRN2")

    # ---- DRAM parameters (per core) ----
    x_in = nc.declare_dram_parameter("x", [BPC, C, N], bf16, isOutput=False)
    wq_d = nc.declare_dram_parameter("wq", [C, C], bf16, isOutput=False)
    wk_d = nc.declare_dram_parameter("wk", [C, C], bf16, isOutput=False)
    wv_d = nc.declare_dram_parameter("wv", [C, C], bf16, isOutput=False)
    wo_d = nc.declare_dram_parameter("wo", [C, C], bf16, isOutput=False)
    w1_d = nc.declare_dram_parameter(
        "w1", [C, F], f8 if F8_FFN1 else bf16, isOutput=False
    )
    w2_d = nc.declare_dram_parameter(
        "w2", [F, C], f8 if F8_FFN2 else bf16, isOutput=False
    )
    bq_d = nc.declare_dram_parameter("bq", [C], f32, isOutput=False)
    bk_d = nc.declare_dram_parameter("bk", [C], f32, isOutput=False)
    bo_d = nc.declare_dram_parameter("bo", [C], f32, isOutput=False)
    b1_d = nc.declare_dram_parameter("b1", [F], f32, isOutput=False)
    wsq_d = nc.declare_dram_parameter("wsq", [C], f32, isOutput=False)
    wsk_d = nc.declare_dram_parameter("wsk", [C], f32, isOutput=False)
    csv_d = nc.declare_dram_parameter("csv", [C], bf16, isOutput=False)
    # relative-position bias, transposed per head, token-chunked
    eb0_d = nc.declare_dram_parameter("eb0", [M0, HEADS, N], bf16, isOutput=False)
    eb1_d = nc.declare_dram_parameter("eb1", [M1, HEADS, N], bf16, isOutput=False)
    id_d = nc.declare_dram_parameter("ident", [128, 128], bf16, isOutput=False)
    out_d = nc.declare_dram_parameter("out", [BPC, C, N], f32, isOutput=True)

    def ld(pool, name, dram, shape, pat):
        t = pool.tile(shape, dram.dtype, name=name)
        nc.sync.dma_start(t[:], dram.ap().rearrange(pat, p=128) if pat else dram.ap())
        return t

    with tile.TileContext(nc) as tc:
        with (
            tc.tile_pool(name="wpool", bufs=1) as wpool,
            tc.tile_pool(name="acts", bufs=1) as acts,
            tc.tile_pool(name="xio", bufs=2) as xio,
            tc.tile_pool(name="small", bufs=2) as small,
            tc.tile_pool(name="psum", bufs=1, space="PSUM") as pp,
        ):
            def load_x(pair):
                b0 = 2 * pair
                t = xio.tile([128, KC, 2, N], bf16, name="xf", tag="xf", bufs=3)
                for b in range(2):
                    nc.sync.dma_start(
                        t[:, :, b, :],
                        x_in.ap()[b0 + b : b0 + b + 1].rearrange(
                            "b (ko p) n -> p ko (b n)", p=128
                        ),
                    )
                return t.rearrange("p k b n -> p k (b n)")

            # ---- DMA issue order = arrival order ----
            xfs = {0: load_x(0)}
            wq_sb = ld(wpool, "wq_sb", wq_d, [128, KC, C], "(ko p) m -> p ko m")
            wk_sb = ld(wpool, "wk_sb", wk_d, [128, KC, C], "(ko p) m -> p ko m")
            bq_sb = ld(wpool, "bq_sb", bq_d, [128, KC], "(mo p) -> p mo")
            bk_sb = ld(wpool, "bk_sb", bk_d, [128, KC], "(mo p) -> p mo")
            wsq_sb = ld(wpool, "wsq_sb", wsq_d, [128, KC], "(mo p) -> p mo")
            wsk_sb = ld(wpool, "wsk_sb", wsk_d, [128, KC], "(mo p) -> p mo")
            wv_sb = ld(wpool, "wv_sb", wv_d, [128, KC, C], "(ko p) m -> p ko m")
            csv_sb = ld(wpool, "csv_sb", csv_d, [1, C], None)
            xfs[1] = load_x(1)
            eb0_sb = ld(wpool, "eb0_sb", eb0_d, [M0, HEADS, N], None)
            eb1_sb = ld(wpool, "eb1_sb", eb1_d, [M1, HEADS, N], None)
            id_sb = ld(wpool, "id_sb", id_d, [128, 128], None)
            wo_sb = ld(wpool, "wo_sb", wo_d, [128, KC, C], "(ko p) m -> p ko m")
            bo_sb = ld(wpool, "bo_sb", bo_d, [128, KC], "(mo p) -> p mo")
            w1_sb = ld(wpool, "w1_sb", w1_d, [128, KC, F], "(ko p) m -> p ko m")
            b1_sb = ld(wpool, "b1_sb", b1_d, [128, KF], "(mo p) -> p mo")
            w2_sb = ld(wpool, "w2_sb", w2_d, [128, KF, C], "(ko p) m -> p ko m")

            ones_sb = wpool.tile([128, 1], bf16, name="ones_sb")
            nc.vector.memset(ones_sb[:], 1.0)
            ones32_sb = wpool.tile([1, 1], f32, name="ones32_sb")
            nc.vector.memset(ones32_sb[:], 1.0)
            onesrow_sb = wpool.tile([1, 64], bf16, name="onesrow_sb")
            nc.vector.memset(onesrow_sb[:], 1.0)
            zero_sb = wpool.tile([128, 1], f32, name="zero_sb")
            nc.vector.memset(zero_sb[:], 0.0)
            # V tiles: data cols 0:64 rewritten per pair, ones half static
            vts = [
                [
                    wpool.tile([128, HEADS, 128], bf16, name=f"vt_{j}_{ci}")
                    for ci in range(2)
                ]
                for j in range(2)
            ]
            for j in range(2):
                for ci in range(2):
                    nc.vector.memset(vts[j][ci][:, :, DH:128], 1.0)

            eb_sb = (eb0_sb, eb1_sb)
            NP2 = BPC // 2

            # persistent-by-tag activation tiles (rewritten per pair / j)
            def a_tile(shape, dt_, name, bufs=1):
                return acts.tile(shape, dt_, name=name, tag=name, bufs=bufs)

            def stage_stats(pair, xsqs):
                """LN stats for `pair`: emits 12 tiny PE matmuls, the fp32
                stats chain, broadcasts and rstd column transposes. xsqs
                were emitted earlier so gpsimd is already done."""
                xf = xfs[pair]
                s_ps = pp.tile([1, NPAIR], f32, name="s_ps", tag="oppA")
                for k in range(KC):
                    nc.tensor.matmul(
                        s_ps[:], ones_sb[:], xf[:, k],
                        start=(k == 0), stop=(k == KC - 1),
                    )
                q_ps = pp.tile([1, NPAIR], f32, name="q_ps", tag="oppB")
                for k in range(KC):
                    nc.tensor.matmul(
                        q_ps[:], ones_sb[:], xsqs[k][:],
                        start=(k == 0), stop=(k == KC - 1),
                    )
                mu = small.tile([1, NPAIR], f32, name="mu", tag="mu", bufs=1)
                nc.vector.tensor_scalar_mul(mu[:], s_ps[:], 1.0 / C)
                var = small.tile([1, NPAIR], f32, name="var", tag="var", bufs=1)
                nc.vector.scalar_tensor_tensor(
                    var[:], mu[:], -1.0, mu[:], OP.mult, OP.mult
                )
                nc.vector.scalar_tensor_tensor(
                    var[:], q_ps[:], 1.0 / C, var[:], OP.mult, OP.add
                )
                # std64 = 64*sqrt(var+eps) = sqrt(4096*var + 4096*eps)
                eps2 = small.tile([1, 1], f32, name="eps2", tag="eps2", bufs=1)
                nc.vector.memset(eps2[:], 1e-5 * WS * WS)
                std = small.tile([1, NPAIR], f32, name="std", tag="std", bufs=1)
                nc.scalar.activation(
                    std[:], var[:], AF.Sqrt, bias=eps2[:], scale=WS * WS
                )
                rstd = small.tile([1, NPAIR], f32, name="rstd", tag="rstd", bufs=1)
                nc.vector.reciprocal_approx_fast(rstd[:], std[:])
                mr = small.tile([1, NPAIR], f32, name="mr", tag="mr", bufs=1)
                nc.vector.scalar_tensor_tensor(
                    mr[:], mu[:], -1.0, rstd[:], OP.mult, OP.mult
                )
                negmu = small.tile([1, NPAIR], bf16, name="negmu", tag="negmu")
                with nc.allow_low_precision(reason="bf16 rank-1 mu term"):
                    nc.vector.tensor_scalar_mul(negmu[:], mu[:], -1.0)
                rstd_b = small.tile(
                    [128, NPAIR], f32, name="rstd_b", tag="rstd_b", bufs=2
                )
                nc.gpsimd.partition_broadcast(rstd_b[:], rstd[:])
                mr_b = small.tile([128, NPAIR], f32, name="mr_b", tag="mr_b", bufs=2)
                nc.gpsimd.partition_broadcast(mr_b[:], mr[:])
                # rstd64 transposed to per-token columns for the V epilogue
                rstdc = [[None, None], [None, None]]
                for j in range(2):
                    for ci, (ms, ml) in enumerate(MCHUNKS):
                        tps = pp.tile([128, 1], f32, name="tps", tag="mm", bufs=2)
                        nc.tensor.matmul(
                            tps[:ml],
                            rstd[0:1, j * N + ms : j * N + ms + ml],
                            ones32_sb[:],
                            is_transpose=True,
                        )
                        rc = small.tile(
                            [128, 1], f32, name=f"rc{j}{ci}", tag=f"rc{j}{ci}",
                            bufs=2,
                        )
                        nc.scalar.activation(rc[:ml], tps[:ml], AF.Copy, bias=0.0)
                        rstdc[j][ci] = rc
                return dict(xf=xf, negmu=negmu, rstd_b=rstd_b, mr_b=mr_b,
                            rstdc=rstdc)

            def stage_qk(ctx):
                """Q/K projections from raw bf16 x; LN applied post-hoc."""
                xf = ctx["xf"]
                rstd_b, mr_b = ctx["rstd_b"], ctx["mr_b"]
                qT = a_tile([128, KC, NPAIR], bf16, "qT")
                kT = a_tile([128, KC, NPAIR], bf16, "kT")
                for dst, w_sb, ws_sb, bias_sb in (
                    (qT, wq_sb, wsq_sb, bq_sb),
                    (kT, wk_sb, wsk_sb, bk_sb),
                ):
                    for m in range(KC):
                        ps = pp.tile([128, NPAIR], f32, name="ps_qk", tag="mm",
                                     bufs=2)
                        for k in range(KC):
                            nc.tensor.matmul(
                                ps[:],
                                w_sb[:, k, 128 * m : 128 * (m + 1)],
                                xf[:, k],
                                start=(k == 0),
                                stop=(k == KC - 1),
                            )
                        t1 = small.tile([128, NPAIR], f32, name="t1", tag="t1")
                        nc.vector.tensor_mul(t1[:], ps[:], rstd_b[:])
                        t2 = small.tile([128, NPAIR], f32, name="t2", tag="t2")
                        nc.vector.scalar_tensor_tensor(
                            t2[:], mr_b[:], ws_sb[:, m : m + 1], t1[:],
                            OP.mult, OP.add,
                        )
                        with nc.allow_low_precision(reason="bf16 qk"):
                            nc.vector.tensor_scalar_add(
                                dst[:, m, :], t2[:], bias_sb[:, m : m + 1]
                            )
                ctx.update(qT=qT, kT=kT)

            def stage_b(pair, ctx):
                b0 = 2 * pair
                xf, qT, kT = ctx["xf"], ctx["qT"], ctx["kT"]
                negmu, rstdc = ctx["negmu"], ctx["rstdc"]
                if pair + 2 < NP2:
                    xfs[pair + 2] = load_x(pair + 2)

                # ---- V projection (token-major) from raw x ----
                def emit_vproj(j, ci, s):
                    ms, ml = MCHUNKS[ci]
                    vt = vts[j][ci]
                    pv = pp.tile([128, 384], f32, name="pv", tag="mm", bufs=2)
                    for k in range(KC):
                        nc.tensor.matmul(
                            pv[:ml],
                            xf[:, k, j * N + ms : j * N + ms + ml],
                            wv_sb[:, k, 384 * s : 384 * (s + 1)],
                            start=(k == 0),
                            stop=False,
                        )
                    nc.tensor.matmul(
                        pv[:ml],
                        negmu[0:1, j * N + ms : j * N + ms + ml],
                        csv_sb[0:1, 384 * s : 384 * (s + 1)],
                        start=False,
                        stop=True,
                    )
                    nc.scalar.activation(
                        vt[:ml, 6 * s : 6 * (s + 1), 0:DH],
                        pv[:ml].rearrange("p (h d) -> p h d", h=6),
                        AF.Identity,
                        bias=zero_sb[:ml],
                        scale=rstdc[j][ci][:ml],
                    )

                OT = a_tile([128, KC, NPAIR], bf16, "OT")
                ybf = a_tile([128, KC, NPAIR], bf16, "ybf")
                yb8 = a_tile([128, KC, NPAIR], f8, "yb8")

                def emit_sims(j, hp):
                    cb = j * N
                    ets = []
                    for hh in range(2):
                        h = 2 * hp + hh
                        prow = 64 * hh
                        sim = pp.tile(
                            [128, 2, N], f32, name="sim",
                            tag="simA" if hh == 0 else "simB", bufs=2,
                        )
                        for ci, (ms, ml) in enumerate(MCHUNKS):
                            # bias add via identity matmul; zero-fills rows
                            # ml:128 so the merged exp reads no stale PSUM
                            nc.tensor.matmul(
                                sim[:, ci],
                                id_sb[:ml, :],
                                eb_sb[ci][:ml, h, :],
                                start=True,
                                stop=False,
                                skip_group_check=True,
                            )
                            nc.tensor.matmul(
                                sim[:ml, ci],
                                kT[prow : prow + 64, hp,
                                   cb + ms : cb + ms + ml],
                                qT[prow : prow + 64, hp, cb : cb + N],
                                start=False,
                                stop=True,
                                skip_group_check=True,
                            )
                        et = small.tile(
                            [128, 2, N], bf16, name="et", tag=f"et{hh}"
                        )
                        nc.scalar.activation(et[:], sim[:], AF.Exp, bias=0.0)
                        ets.append(et)
                    return ets

                def emit_av(j, hp, ets):
                    opp = pp.tile(
                        [128, 2, N], f32, name="opp",
                        tag="oppA" if hp % 2 == 0 else "oppB",
                    )
                    for hh in range(2):
                        h = 2 * hp + hh
                        for ci, (ms, ml) in enumerate(MCHUNKS):
                            nc.tensor.matmul(
                                opp[:, hh],
                                vts[j][ci][:ml, h, :],
                                ets[hh][:ml, ci],
                                start=(ci == 0),
                                stop=(ci == 1),
                            )
                    return opp

                def emit_epi(j, hp, opp):
                    cb = j * N
                    rps = pp.tile([128, N], f32, name="rps", tag="mm", bufs=2)
                    # custom-DVE reciprocal can't read PSUM: bounce the two
                    # denominator rows through SBUF in one scalar copy
                    den2 = small.tile([1, 2, N], f32, name="den2", tag="den2")
                    nc.scalar.activation(
                        den2[:], opp[DH : DH + 1, :, :], AF.Copy, bias=0.0
                    )
                    for hh in range(2):
                        rec = small.tile([1, N], f32, name="rec", tag=f"rec{hh}")
                        nc.vector.reciprocal_approx_fast(
                            rec[:], den2[:, hh, :]
                        )
                        rb = small.tile([1, N], bf16, name="rb", tag=f"rb{hh}")
                        with nc.allow_low_precision(reason="bf16 softmax recip"):
                            nc.vector.tensor_copy(rb[:], rec[:])
                        nc.tensor.matmul(
                            rps[64 * hh : 64 * hh + 64, :],
                            onesrow_sb[:], rb[:],
                            start=True, stop=True,
                        )
                    rsb = small.tile([128, N], bf16, name="rsb", tag="rsb")
                    nc.scalar.activation(rsb[:], rps[:], AF.Copy, bias=0.0)
                    for hh in range(2):
                        with nc.allow_low_precision(reason="bf16 attn out"):
                            nc.vector.tensor_mul(
                                OT[64 * hh : 64 * hh + 64, hp, cb : cb + N],
                                opp[0:DH, hh],
                                rsb[64 * hh : 64 * hh + 64, :],
                            )

                def attention_j(j, fillers):
                    # 2-deep software pipeline: sims(hp) | av(hp-1) | epi(hp-2)
                    fq = list(fillers)
                    if not _DO_ATTN:
                        nc.vector.memset(OT[:, :, j * N : j * N + N], 0.01)
                        while fq:
                            fq.pop(0)()
                        return

                    def fill():
                        if fq:
                            fq.pop(0)()

                    pend_ets = []  # [(hp, ets)] awaiting attn@v
                    pend_opp = []  # [(hp, opp)] awaiting epilogue

                    def step_av():
                        php, pets = pend_ets.pop(0)
                        opp = emit_av(j, php, pets)
                        if pend_opp:
                            ehp, eopp = pend_opp.pop(0)
                            emit_epi(j, ehp, eopp)
                        pend_opp.append((php, opp))

                    for hp in range(HEADS // 2):
                        ets = emit_sims(j, hp)
                        fill()
                        if pend_ets:
                            step_av()
                        pend_ets.append((hp, ets))
                    step_av()
                    while pend_opp:
                        ehp, eopp = pend_opp.pop(0)
                        emit_epi(j, ehp, eopp)
                    while fq:
                        fq.pop(0)()

                def emit_outproj(j, m):
                    cb = j * N
                    po = pp.tile([128, N], f32, name="po", tag="mm", bufs=2)
                    for k in range(KC):
                        nc.tensor.matmul(
                            po[:],
                            wo_sb[:, k, 128 * m : 128 * (m + 1)],
                            OT[:, k, cb : cb + N],
                            start=(k == 0),
                            stop=(k == KC - 1),
                        )
                    with nc.allow_low_precision(reason="bf16 residual-1"):
                        nc.vector.scalar_tensor_tensor(
                            ybf[:, m, cb : cb + N],
                            po[:],
                            bo_sb[:, m : m + 1],
                            xf[:, m, cb : cb + N],
                            OP.add, OP.add,
                        )
                    with nc.allow_low_precision(reason="fp8 ffn input"):
                        nc.vector.tensor_copy(
                            yb8[:, m, cb : cb + N], ybf[:, m, cb : cb + N]
                        )

                o32 = xio.tile([128, 2, KC, N], f32, name="o32", tag="o32",
                               bufs=1)

                def ffn_core_j(j):
                    cb = j * N
                    h1t = a_tile([128, KF, N], f8 if F8_FFN2 else bf16, "h1s")
                    tags1 = ("simA", "simB", "oppA")
                    tags2 = ("simA", "simB", "oppB")
                    p_a = [
                        pp.tile([128, N], f32, name=f"pfa{o}", tag=tags1[o], bufs=2 if tags1[o] in ("simA", "simB") else 1)
                        for o in range(3)
                    ]

                    def h1_chunk(mf):
                        p1 = pp.tile([128, N], f32, name="p1", tag="mm",
                                     bufs=2)
                        if F8_FFN1:
                            for kk in range(KC // 2):
                                nc.tensor.matmul(
                                    p1[:],
                                    w1_sb[:, 2 * kk : 2 * kk + 2,
                                          128 * mf : 128 * (mf + 1)],
                                    yb8[:, 2 * kk : 2 * kk + 2, cb : cb + N],
                                    start=(kk == 0),
                                    stop=(kk == KC // 2 - 1),
                                    perf_mode=DR,
                                )
                        else:
                            for k in range(KC):
                                nc.tensor.matmul(
                                    p1[:],
                                    w1_sb[:, k, 128 * mf : 128 * (mf + 1)],
                                    ybf[:, k, cb : cb + N],
                                    start=(k == 0),
                                    stop=(k == KC - 1),
                                )
                        nc.scalar.activation(
                            h1t[:, mf, :], p1[:],
                            AF.Identity if _SIM_NO_GELU else AF.Gelu,
                            bias=b1_sb[:, mf : mf + 1],
                            scale=1.0 / WS,
                        )

                    if F8_FFN2:
                        for t in range(KF // 2):
                            h1_chunk(2 * t)
                            h1_chunk(2 * t + 1)
                            for o in range(3):
                                nc.tensor.matmul(
                                    p_a[o][:],
                                    w2_sb[:, 2 * t : 2 * t + 2,
                                          128 * o : 128 * (o + 1)],
                                    h1t[:, 2 * t : 2 * t + 2, :],
                                    start=(t == 0),
                                    stop=(t == KF // 2 - 1),
                                    perf_mode=DR,
                                )
                        p_b = [
                            pp.tile([128, N], f32, name=f"pfb{o}", tag=tags2[o], bufs=2 if tags2[o] in ("simA", "simB") else 1)
                            for o in range(3)
                        ]
                        for o in range(3):
                            for t in range(KF // 2):
                                nc.tensor.matmul(
                                    p_b[o][:],
                                    w2_sb[:, 2 * t : 2 * t + 2,
                                          128 * (o + 3) : 128 * (o + 4)],
                                    h1t[:, 2 * t : 2 * t + 2, :],
                                    start=(t == 0),
                                    stop=(t == KF // 2 - 1),
                                    perf_mode=DR,
                                )
                    else:
                        for mf in range(KF):
                            h1_chunk(mf)
                            for o in range(3):
                                nc.tensor.matmul(
                                    p_a[o][:],
                                    w2_sb[:, mf, 128 * o : 128 * (o + 1)],
                                    h1t[:, mf, :],
                                    start=(mf == 0),
                                    stop=(mf == KF - 1),
                                )
                        p_b = [
                            pp.tile([128, N], f32, name=f"pfb{o}", tag=tags2[o], bufs=2 if tags2[o] in ("simA", "simB") else 1)
                            for o in range(3)
                        ]
                        for o in range(3):
                            for mf in range(KF):
                                nc.tensor.matmul(
                                    p_b[o][:],
                                    w2_sb[:, mf, 128 * (o + 3) : 128 * (o + 4)],
                                    h1t[:, mf, :],
                                    start=(mf == 0),
                                    stop=(mf == KF - 1),
                                )
                    return p_a + p_b

                def ffn_store_j(j, pps):
                    cb = j * N
                    for o in range(KC):
                        nc.vector.scalar_tensor_tensor(
                            o32[:, j, o, :],
                            pps[o][:],
                            1.0 / WS,
                            ybf[:, o, cb : cb + N],
                            OP.mult, OP.add,
                        )
                        nc.sync.dma_start(
                            out_d.ap()[
                                b0 + j : b0 + j + 1, 128 * o : 128 * (o + 1)
                            ].rearrange("b (ko p) n -> p (b ko n)", p=128),
                            o32[:, j, o, :],
                        )

                # ---- emission schedule for the pair ----
                for ci in range(2):
                    for s in range(2):
                        emit_vproj(0, ci, s)
                attention_j(
                    0,
                    [lambda ci=ci, s=s: emit_vproj(1, ci, s)
                     for ci in range(2) for s in range(2)],
                )
                attention_j(
                    1,
                    [lambda m=m: emit_outproj(0, m) for m in range(KC)],
                )
                # squares for next pair's stats: gpsimd crunches during FFN
                xsqs = None
                if pair + 1 < NP2:
                    xfn = xfs[pair + 1]
                    xsqs = []
                    for k in range(KC):
                        xsq = small.tile(
                            [128, NPAIR], bf16, name="xsq", tag="xsq", bufs=6
                        )
                        nc.gpsimd.tensor_mul(xsq[:], xfn[:, k], xfn[:, k])
                        xsqs.append(xsq)
                if _DO_FFN:
                    pps0 = ffn_core_j(0)
                    ffn_store_j(0, pps0)
                else:
                    for o in range(KC):
                        nc.vector.tensor_copy(o32[:, 0, o, :], ybf[:, o, 0:N])
                        nc.sync.dma_start(
                            out_d.ap()[b0 : b0 + 1, 128 * o : 128 * (o + 1)]
                            .rearrange("b (ko p) n -> p (b ko n)", p=128),
                            o32[:, 0, o, :])
                nctx = None
                if _SPLICE:
                    nctx = (stage_stats(pair + 1, xsqs)
                            if pair + 1 < NP2 else None)
                for m in range(KC):
                    emit_outproj(1, m)
                if nctx is not None:
                    stage_qk(nctx)
                if _DO_FFN:
                    pps1 = ffn_core_j(1)
                    ffn_store_j(1, pps1)
                else:
                    for o in range(KC):
                        nc.vector.tensor_copy(o32[:, 1, o, :], ybf[:, o, N:])
                        nc.sync.dma_start(
                            out_d.ap()[b0 + 1 : b0 + 2, 128 * o : 128 * (o + 1)]
                            .rearrange("b (ko p) n -> p (b ko n)", p=128),
                            o32[:, 1, o, :])
                if not _SPLICE and pair + 1 < NP2:
                    nctx = stage_stats(pair + 1, xsqs)
                    stage_qk(nctx)
                if pair + 1 < NP2:
                    xfs.pop(pair)
                return nctx

            # pair 0 prologue: squares first so gpsimd leads the stats
            xsqs0 = []
            for k in range(KC):
                xsq = small.tile(
                    [128, NPAIR], bf16, name="xsq", tag="xsq", bufs=6
                )
                nc.gpsimd.tensor_mul(xsq[:], xfs[0][:, k], xfs[0][:, k])
                xsqs0.append(xsq)
            ctx = stage_stats(0, xsqs0)
            stage_qk(ctx)
            for pair in range(NP2):
                ctx = stage_b(pair, ctx)
    nc.finalize()
    return nc


_CACHE = {}


def prepare_in_maps(inputs):
    x = np.asarray(inputs["x"], dtype=np.float32)  # (64, 768, 14, 14)
    ln_g = np.asarray(inputs["ln_g"], dtype=np.float32)
    ln_b = np.asarray(inputs["ln_b"], dtype=np.float32)
    wq = np.asarray(inputs["wq"], dtype=np.float32)
    bq = np.asarray(inputs["bq"], dtype=np.float32)
    wk = np.asarray(inputs["wk"], dtype=np.float32)
    bk = np.asarray(inputs["bk"], dtype=np.float32)
    wv = np.asarray(inputs["wv"], dtype=np.float32)
    bv = np.asarray(inputs["bv"], dtype=np.float32)
    wo = np.asarray(inputs["wo"], dtype=np.float32)
    bo = np.asarray(inputs["bo"], dtype=np.float32)
    w1 = np.asarray(inputs["w1"], dtype=np.float32)
    b1 = np.asarray(inputs["b1"], dtype=np.float32)
    w2 = np.asarray(inputs["w2"], dtype=np.float32)
    b2 = np.asarray(inputs["b2"], dtype=np.float32)
    rel_bias = np.asarray(inputs["rel_bias"], dtype=np.float32)

    bf = ml_dtypes.bfloat16
    f8 = ml_dtypes.float8_e4m3  # TRN E4M3 (max 240, has inf)

    # Fold LayerNorm gamma into QKV weights, beta into their biases;
    # pre-scale QKV weights by WS=64 (exact in bf16) so one rstd/64
    # serves every epilogue.
    wqp = (WS * ln_g[:, None] * wq).astype(bf)
    wkp = (WS * ln_g[:, None] * wk).astype(bf)
    wvp = (WS * ln_g[:, None] * wv).astype(bf)
    bqp = (ln_b @ wq + bq).astype(np.float32)
    bkp = (ln_b @ wk + bk).astype(np.float32)
    bvp = (ln_b @ wv + bv).astype(np.float32)
    # V bias commutes through softmax; fold into out-proj bias along
    # with b2 (the FFN1 bias is corrected to compensate).
    bop = (bo + bvp @ wo + b2).astype(np.float32)
    b1p = (b1 - b2 @ w1).astype(np.float32)
    # column sums of the exact on-chip weights for the LN mean correction
    wsq = wqp.astype(np.float32).sum(axis=0).astype(np.float32)
    wsk = wkp.astype(np.float32).sum(axis=0).astype(np.float32)
    csv = wvp.astype(np.float32).sum(axis=0).astype(bf)

    # FFN weights, pre-scaled x64, fp8 when enabled
    if F8_FFN1:
        w1p = np.clip(WS * w1, -240.0, 240.0).astype(f8)
    else:
        w1p = (WS * w1).astype(bf)
    if F8_FFN2:
        w2p = np.clip(WS * w2, -240.0, 240.0).astype(f8)
    else:
        w2p = (WS * w2).astype(bf)

    # Relative position bias, transposed per head (raw, added in PSUM)
    rel_idx = _relative_indices()
    bias = rel_bias[:, rel_idx]  # (HEADS, N, N) : bias[h, n, m]
    eb_m = bias.transpose(2, 0, 1)  # (m, HEADS, n)
    eb0 = np.ascontiguousarray(eb_m[:M0]).astype(bf)
    eb1 = np.ascontiguousarray(eb_m[M0:]).astype(bf)

    ident = np.eye(128, dtype=np.float32).astype(bf)

    common = {
        "wq": wqp, "wk": wkp, "wv": wvp, "wo": wo.astype(bf),
        "w1": w1p, "w2": w2p,
        "bq": bqp, "bk": bkp, "bo": bop, "b1": b1p,
        "wsq": wsq, "wsk": wsk, "csv": csv,
        "eb0": eb0, "eb1": eb1,
        "ident": ident,
    }

    x_flat = x.reshape(B, C, N)
    in_maps = []
    for c in range(NCORES):
        m = dict(common)
        m["x"] = np.ascontiguousarray(x_flat[c * BPC : (c + 1) * BPC]).astype(bf)
        in_maps.append(m)
    return in_maps


def kernel(**inputs):
    import sys

    if "/opt/trn_rl_repo" not in sys.path:
        sys.path.insert(0, "/opt/trn_rl_repo")
    from concourse.bass_utils import run_bass_kernel_spmd

    in_maps = prepare_in_maps(inputs)

    if "nc" not in _CACHE:
        _CACHE["nc"] = _build_bass()
    nc = _CACHE["nc"]

    res = run_bass_kernel_spmd(nc, in_maps, core_ids=list(range(NCORES)))
    _CACHE["last_res"] = res
    outs = [r["out"] for r in res.results]
    full = np.concatenate(outs, axis=0)  # (64, 768, 196)
    return full.reshape(B, C, H, W).astype(np.float32)
